# revision 35
# baseline (speedup 1.0000x reference)
# Multi-head attention (B=2, S=2048, D=1024, H=16) on 8 TRN2 NeuronCores.
#
# Sharding: core c -> batch b = c//4, head group g = c%4 (4 heads = 256
# features). Each core computes its heads' attention for its batch plus the
# row-parallel slice of the output projection; the host sums the 4 partials
# per batch (the all-reduce) and adds bo.
#
# Device math per core (layouts transposed so softmax needs no cross-
# partition reduce; all matmul operands fp16, accumulation fp32 in PSUM):
#   qhT[f, s] = wq_g @ q_b^T ; khT likewise
#   vh[k, slot] : v-projection, slot layout per head [1 | 0pad63 | v 64]
#   logitsT[k, q] = khT_h^T @ qhT_h     (K=64 halves at partition offsets
#                                        0/64 -> PE tile-concurrent pairs)
#   ex = exp(logitsT/8); ex *= (1-mask)^T in place     (ACT then DVE)
#   av_h[slot, q] = vh^T @ ex           (row 0 = denominator, 64:128 = out)
#   attnN16[pair][128, q] = packed av pair * (1/denom broadcast across
#       partitions via SBUF->SBUF DMA of the reciprocal row)
#   partial[q, D] = attnN16^T @ wo_pair (K=128 pair-packed matmuls)
#
# Emission order: q proj (c-outer, all 8 PSUM banks), v proj, then 4
# attention blocks; k proj (qs-outer) is interleaved into block 0 so exp
# starts as early as possible.  Per steady block: WO of the previous block
# in slots kc3..6, AV pops 2/kc from kc8, mask prefetch at kc8/12,
# normalization at the block boundary.
import os
import numpy as np

B, S, DM, H, DEPTH = 2, 2048, 1024, 16, 64
NCORES = 8
GROUPS = 4            # head-groups per batch == cores per batch
HG = H // GROUPS      # heads per core
FS = HG * DEPTH       # features per core
QC = 512              # q-block (matmul free dim)
NQC = S // QC
NKC = S // 128        # k chunks
PAIRS = HG // 2
CCH = DM // 128       # contraction chunks for the projections

_CACHE = {}


def _build():
    import concourse.tile as tile
    from concourse import bacc, mybir

    dt = mybir.dt
    f32, f16 = dt.float32, dt.float16
    Act = mybir.ActivationFunctionType

    nc = bacc.Bacc("TRN2", target_bir_lowering=False, debug=False,
                   num_devices=NCORES)

    xq = nc.dram_tensor("xq", [DM, S], f16, kind="ExternalInput").ap()
    xk = nc.dram_tensor("xk", [DM, S], f16, kind="ExternalInput").ap()
    xv = nc.dram_tensor("xv", [DM, S], f16, kind="ExternalInput").ap()
    wqd = nc.dram_tensor("wq", [DM, FS], f16, kind="ExternalInput").ap()
    wkd = nc.dram_tensor("wk", [DM, FS], f16, kind="ExternalInput").ap()
    wvd = nc.dram_tensor("wv", [DM, FS], f16, kind="ExternalInput").ap()
    wod = nc.dram_tensor("wo", [PAIRS, 128, DM], f16, kind="ExternalInput").ap()
    m01 = nc.dram_tensor("m01", [S, S], f16, kind="ExternalInput").ap()
    bqd = nc.dram_tensor("bq", [128, 2], f32, kind="ExternalInput").ap()
    bkd = nc.dram_tensor("bk", [128, 2], f32, kind="ExternalInput").ap()
    out = nc.dram_tensor("part", [S, DM], f32, kind="ExternalOutput").ap()

    with tile.TileContext(nc) as tc:
        with (
            tc.tile_pool(name="xq", bufs=3) as xqp,       # q-proj x transient
            tc.tile_pool(name="xr", bufs=16) as xrp,      # xk + xv resident
            tc.tile_pool(name="wp", bufs=16) as wp,       # weight chunks
            tc.tile_pool(name="wop", bufs=1) as wop,
            tc.tile_pool(name="qk", bufs=4) as qkp,
            tc.tile_pool(name="vh", bufs=16) as vp,
            tc.tile_pool(name="mk", bufs=2) as mkp,       # mask half-tiles
            tc.tile_pool(name="ex", bufs=18) as exp_p,
            tc.tile_pool(name="an", bufs=4) as anp,
            tc.tile_pool(name="rr", bufs=2) as rrp,
            tc.tile_pool(name="rb", bufs=3) as rbp,
            tc.tile_pool(name="os", bufs=3) as osp,
            tc.tile_pool(name="cst", bufs=4) as cst,
            tc.tile_pool(name="ps2", bufs=2, space="PSUM") as ps2,   # 2-bank
            tc.tile_pool(name="ps1", bufs=4, space="PSUM") as ps1,   # 1-bank
        ):
            def big():
                return ps2.tile([128, 2, QC], f32, tag="big", name="big")

            def bank():
                return ps1.tile([128, QC], f32, tag="bank", name="bank")

            # ---- input DMAs ----
            # sync queue carries the q/k critical path (plus in-loop masks
            # and output stores); the scalar HWDGE queue carries the v/wo/
            # mask0 stream concurrently.
            def chunk_dma(pool, tag, dram, c, eng):
                t = pool.tile([128, S], f16, tag=tag, name=tag)
                eng.dma_start(t[:], dram[128 * c:128 * (c + 1), :])
                return t

            # mask half-tiles: [128, 8, QC] covering kc 8*h .. 8*h+7
            def mask_dma(qcb, h, eng):
                t = mkp.tile([128, NKC // 2, QC], f16, tag="mk", name="mk")
                eng.dma_start(
                    t[:],
                    m01[S // 2 * h:S // 2 * (h + 1), QC * qcb:QC * (qcb + 1)]
                    .rearrange("(kc p) q -> p kc q", p=128))
                return t

            wq_t, wk_t, xk_t, xv_t = [], [], [], []
            # sync stream: q path (weights first -- the first matmul needs
            # wq c0 + xq c0 only)
            w0 = wp.tile([128, FS], f16, tag="w", name="w")
            nc.sync.dma_start(w0[:], wqd[0:128, :])
            wq_t.append(w0)
            xq_pend = [chunk_dma(xqp, "xq", xq, c, nc.sync) for c in range(2)]
            for c in range(1, CCH):
                wt = wp.tile([128, FS], f16, tag="w", name="w")
                nc.sync.dma_start(wt[:], wqd[128 * c:128 * (c + 1), :])
                wq_t.append(wt)
            bq_t = cst.tile([128, 2], f32, tag="bias", name="bias")
            nc.sync.dma_start(bq_t[:], bqd[:])
            # gpsimd SWDGE stream: k path (its ~1.3us per-issue rate lands
            # these after the q path without stealing early bandwidth);
            # mask halves slot in so h0 arrives just before the first exp
            mk_cur = [None, None]
            mk_next = [None, None]
            for ck in range(CCH):
                wt = wp.tile([128, FS], f16, tag="w", name="w")
                nc.gpsimd.dma_start(wt[:], wkd[128 * ck:128 * (ck + 1), :])
                wk_t.append(wt)
                xk_t.append(chunk_dma(xrp, "xr", xk, ck, nc.gpsimd))
                if ck == 3:
                    mk_cur[0] = mask_dma(0, 0, nc.gpsimd)
            bk_t = cst.tile([128, 2], f32, tag="bias", name="bias")
            nc.gpsimd.dma_start(bk_t[:], bkd[:])
            mk_cur[1] = mask_dma(0, 1, nc.gpsimd)
            # scalar-queue stream: the v path + wo
            wv_t = wp.tile([128, CCH, FS], f16, tag="wv", name="wv", bufs=1)
            nc.scalar.dma_start(wv_t[:], wvd.rearrange("(c p) f -> p c f",
                                                       p=128))
            for c in range(CCH):
                xv_t.append(chunk_dma(xrp, "xr", xv, c, nc.scalar))
            wo_t = wop.tile([128, PAIRS, DM], f16, tag="wo", name="wo")
            nc.scalar.dma_start(wo_t[:], wod.rearrange("h p d -> p h d"))

            # ---- q projection (c-outer; transposed [FS, S] fp16 out) ----
            qhT = [qkp.tile([128, S], f16, tag="qk", name="qk")
                   for _ in range(2)]
            khT = [qkp.tile([128, S], f16, tag="qk", name="qk")
                   for _ in range(2)]

            # accumulators: 2 big (qs 0,1 x m) + 4 bank (qs 2,3 x m)
            pb = [big(), big()]
            ps = [bank() for _ in range(4)]

            def q_target(m, qs):
                if qs < 2:
                    return pb[qs][:, m, :]
                return ps[2 * (qs - 2) + m][:]

            for c in range(CCH):
                xt = xq_pend[c] if c < 2 else chunk_dma(xqp, "xq", xq, c,
                                                        nc.sync)
                for m in range(2):
                    for qs in range(NQC):
                        nc.tensor.matmul(
                            q_target(m, qs),
                            lhsT=wq_t[c][:, 128 * m:128 * (m + 1)],
                            rhs=xt[:, QC * qs:QC * (qs + 1)],
                            start=(c == 0), stop=(c == CCH - 1))
            for qs in range(NQC):
                for m in range(2):
                    nc.scalar.add(
                        qhT[m][:, QC * qs:QC * (qs + 1)],
                        q_target(m, qs), bq_t[:, m:m + 1])

            # ---- v projection, interleaved into block 0 (see below) ----
            vh = []

            def proj_v_kr(kr):
                pv = bank()
                for c in range(CCH):
                    nc.tensor.matmul(
                        pv[:, 0:FS],
                        lhsT=xv_t[c][:, 128 * kr:128 * (kr + 1)],
                        rhs=wv_t[:, c, :],
                        start=(c == 0), stop=(c == CCH - 1))
                t = vp.tile([128, HG, 128], f16, tag="vh", name="vh")
                nc.vector.memset(t[:, :, 0:1], 1.0)
                nc.vector.memset(t[:, :, 1:64], 0.0)
                nc.vector.tensor_copy(
                    t[:, :, 64:128],
                    pv[:, 0:FS].rearrange("p (h d) -> p h d", d=DEPTH))
                vh.append(t)

            # ---- attention ----
            def proj_k_qs(qs):
                p = big()
                for c in range(CCH):
                    for m in range(2):
                        nc.tensor.matmul(
                            p[:, m, :],
                            lhsT=wk_t[c][:, 128 * m:128 * (m + 1)],
                            rhs=xk_t[c][:, QC * qs:QC * (qs + 1)],
                            start=(c == 0), stop=(c == CCH - 1))
                for m in range(2):
                    nc.vector.tensor_scalar_add(
                        khT[m][:, QC * qs:QC * (qs + 1)],
                        p[:, m, :], bk_t[:, m:m + 1])

            def emit_logits(qcb, kc, pair):
                qsl = slice(QC * qcb, QC * (qcb + 1))
                ksl = slice(128 * kc, 128 * (kc + 1))
                lg2 = big()
                for half in range(2):
                    psl = slice(64 * half, 64 * (half + 1))
                    nc.tensor.matmul(
                        lg2[:, half, :],
                        lhsT=khT[pair][psl, ksl],
                        rhs=qhT[pair][psl, qsl],
                        start=True, stop=True)
                ex2 = exp_p.tile([128, 2, QC], f16, tag="ex", name="ex")
                nc.scalar.activation(ex2[:], lg2[:], Act.Exp, scale=0.125)
                nc.vector.tensor_mul(
                    ex2[:], ex2[:],
                    mk_cur[kc // 8][:, kc % 8:kc % 8 + 1, :]
                    .to_broadcast((128, 2, QC)))
                return ex2

            def emit_av(av2, pair, dk, ex2):
                # av2: [tile_half0, tile_half1]; vh head = 2*pair+half
                for half in range(2):
                    nc.tensor.matmul(
                        av2[half][:],
                        lhsT=vh[dk][:, 2 * pair + half, :],
                        rhs=ex2[:, half, :],
                        start=(dk == 0), stop=(dk == NKC - 1),
                        skip_group_check=True)

            def normalize_pair(av2, p, last=False):
                # drain a pair's av PSUM banks fast, then broadcast 1/denom
                # across partitions on the (otherwise idle) GpSimd.  On the
                # final block the copies go to ACT (its exp stream is done).
                cpy = nc.scalar.copy if last else nc.vector.tensor_copy
                at = anp.tile([128, QC], f16, tag="an", name="an")
                rt = rrp.tile([1, 2, QC], f32, tag="rr", name="rr")
                for half in range(2):
                    cpy(at[64 * half:64 * (half + 1), :],
                        av2[half][64:128, :])
                    nc.vector.reciprocal_approx_fast(
                        rt[0:1, half, :], av2[half][0:1, :])
                rba = rbp.tile([128, QC], f32, tag="rb", name="rb")
                rbb = rbp.tile([128, QC], f32, tag="rb", name="rb")
                nc.gpsimd.partition_broadcast(rba[:], rt[0:1, 0, :])
                nc.gpsimd.partition_broadcast(rbb[:], rt[0:1, 1, :])
                nc.vector.tensor_mul(at[0:64, :], at[0:64, :],
                                     rba[0:64, :])
                nc.vector.tensor_mul(at[64:128, :], at[64:128, :],
                                     rbb[64:128, :])
                return at

            def emit_wo(pq, attnN, qm):
                row = slice(128 * (4 * pq + qm), 128 * (4 * pq + qm + 1))
                po = [bank(), bank()]
                for dn in range(2):
                    for p in range(PAIRS):
                        nc.tensor.matmul(
                            po[dn][:],
                            lhsT=attnN[p][:, 128 * qm:128 * (qm + 1)],
                            rhs=wo_t[:, p, 512 * dn:512 * (dn + 1)],
                            start=(p == 0), stop=(p == PAIRS - 1))
                ot = osp.tile([128, 2, QC], f32, tag="os", name="os")
                nc.scalar.copy(ot[:, 0, :], po[0][:])
                nc.vector.tensor_copy(ot[:, 1, :], po[1][:])
                nc.scalar.dma_start(
                    out[row, :].rearrange("p (o q) -> p o q", o=2), ot[:])

            # v-projection schedule inside block 0: kr chunks per kc slot
            # (xv lands on the scalar queue while q/k stream on sync)
            vsched = {3: 3, 4: 3, 5: 3, 6: 3, 7: 2, 8: 2}

            # ---- block 0: kc-major (k/v projections share the slots) ----
            prev = None
            av_h = None
            pend = {p: [] for p in range(PAIRS)}
            for kc in range(NKC):
                if kc % 4 == 0:
                    proj_k_qs(kc // 4)
                if kc >= 9:                   # after pv has left PSUM
                    if av_h is None:
                        av_h = [bank() for _ in range(HG)]
                    for _ in range(2):
                        for pair in range(PAIRS):
                            if pend[pair]:
                                dk, dex = pend[pair].pop(0)
                                emit_av(av_h[2 * pair:2 * pair + 2],
                                        pair, dk, dex)
                for pair in range(PAIRS):
                    ex2 = emit_logits(0, kc, pair)
                    pend[pair].append((kc, ex2))
                if kc in vsched:
                    for _ in range(vsched[kc]):
                        proj_v_kr(len(vh))
                if kc == 8:
                    mk_next[0] = mask_dma(1, 0, nc.sync)
                elif kc == 12:
                    mk_next[1] = mask_dma(1, 1, nc.sync)
            for pair in range(PAIRS):
                for dk, dex in pend[pair]:
                    emit_av(av_h[2 * pair:2 * pair + 2], pair, dk, dex)
            prev = (0, [normalize_pair(av_h[2 * p:2 * p + 2], p)
                        for p in range(PAIRS)])
            mk_cur = mk_next
            mk_next = [None, None]

            # ---- blocks 1..3: kc-major with AV pops trailing by ~7 ----
            for qcb in range(1, NQC):
                last = qcb == NQC - 1
                av_h = None
                pend = {p: [] for p in range(PAIRS)}
                for kc in range(NKC):
                    if kc >= 7:
                        if av_h is None:
                            av_h = [bank() for _ in range(HG)]
                        for _ in range(2):
                            for pair in range(PAIRS):
                                if pend[pair]:
                                    dk, dex = pend[pair].pop(0)
                                    emit_av(av_h[2 * pair:2 * pair + 2],
                                            pair, dk, dex)
                    for pair in range(PAIRS):
                        ex2 = emit_logits(qcb, kc, pair)
                        pend[pair].append((kc, ex2))
                    if 3 <= kc <= 6 and prev is not None:
                        pq, pattn = prev
                        emit_wo(pq, pattn, kc - 3)
                        if kc == 6:
                            prev = None
                    if not last:
                        if kc == 8:
                            mk_next[0] = mask_dma(qcb + 1, 0, nc.sync)
                        elif kc == 12:
                            mk_next[1] = mask_dma(qcb + 1, 1, nc.sync)
                for pair in range(PAIRS):
                    for dk, dex in pend[pair]:
                        emit_av(av_h[2 * pair:2 * pair + 2], pair, dk, dex)
                prev = (qcb, [normalize_pair(av_h[2 * p:2 * p + 2], p,
                                             last=last)
                              for p in range(PAIRS)])
                mk_cur = mk_next
                mk_next = [None, None]

            pq, attnN = prev
            for qm in range(4):
                emit_wo(pq, attnN, qm)

    nc.compile()
    return nc


def _get_program():
    if "nc" not in _CACHE:
        _CACHE["nc"] = _build()
    return _CACHE["nc"]


def _in_maps(q, k, v, mask, wq, bq, wk, bk, wv, bv, wo, bo):
    q = np.asarray(q, np.float32)
    k = np.asarray(k, np.float32)
    v = np.asarray(v, np.float32)
    mask = np.asarray(mask, np.float32)
    wq = np.asarray(wq, np.float32)
    wk = np.asarray(wk, np.float32)
    wv = np.asarray(wv, np.float32)
    wo = np.asarray(wo, np.float32)
    bq = np.asarray(bq, np.float32)
    bk = np.asarray(bk, np.float32)
    bv = np.asarray(bv, np.float32)
    assert np.all(bv == 0.0), "nonzero bv not supported by this kernel"

    maps = []
    xqT = [np.ascontiguousarray(q[b].T).astype(np.float16) for b in range(B)]
    xkT = [np.ascontiguousarray(k[b].T).astype(np.float16) for b in range(B)]
    xvT = [np.ascontiguousarray(v[b].T).astype(np.float16) for b in range(B)]
    m01 = [np.ascontiguousarray((1.0 - mask[b, 0]).T).astype(np.float16)
           for b in range(B)]
    for c in range(NCORES):
        b, g = divmod(c, GROUPS)
        cols = slice(FS * g, FS * (g + 1))
        # wo rows for this group's heads, pair-packed: [pair, (half, depth), DM]
        wog = np.ascontiguousarray(
            wo[:, cols].T.reshape(PAIRS, 128, DM)).astype(np.float16)
        maps.append({
            "xq": xqT[b], "xk": xkT[b], "xv": xvT[b],
            "wq": np.ascontiguousarray(wq[cols].T).astype(np.float16),
            "wk": np.ascontiguousarray(wk[cols].T).astype(np.float16),
            "wv": np.ascontiguousarray(wv[cols].T).astype(np.float16),
            "wo": wog,
            "m01": m01[b],
            "bq": np.ascontiguousarray(bq[cols].reshape(2, 128).T),
            "bk": np.ascontiguousarray(bk[cols].reshape(2, 128).T),
        })
    return maps


def _run(maps, trace=False):
    from concourse.bass_utils import run_bass_kernel_spmd
    nc = _get_program()
    kwargs = {}
    if trace:
        kwargs = dict(trace=True, tmpdir=os.environ.get("KERNEL_TRACE_DIR"))
    return run_bass_kernel_spmd(nc, maps, list(range(NCORES)), **kwargs)


def kernel(q, k, v, mask, wq, bq, wk, bk, wv, bv, wo, bo):
    maps = _in_maps(q, k, v, mask, wq, bq, wk, bk, wv, bv, wo, bo)
    res = _run(maps)
    parts = [res.results[c]["part"] for c in range(NCORES)]
    bo = np.asarray(bo, np.float32)
    outb = [parts[GROUPS * b] + parts[GROUPS * b + 1]
            + parts[GROUPS * b + 2] + parts[GROUPS * b + 3] + bo
            for b in range(B)]
    return np.stack(outb, 0).astype(np.float32)


# revision 36
# speedup vs baseline: 1.1132x; 1.1132x over previous
# Multi-head attention (B=2, S=2048, D=1024, H=16) on 8 TRN2 NeuronCores.
#
# Sharding: core c -> batch b = c//4, head group g = c%4 (4 heads = 256
# features). Each core computes its heads' attention for its batch plus the
# row-parallel slice of the output projection; the host sums the 4 partials
# per batch (the all-reduce) and adds bo.
#
# Device math per core (layouts transposed so softmax needs no cross-
# partition reduce; all matmul operands fp16, accumulation fp32 in PSUM):
#   qhT[f, s] = wq_g @ q_b^T ; khT likewise
#   vh[k, slot] : v-projection, slot layout per head [1 | 0pad63 | v 64]
#   logitsT[k, q] = khT_h^T @ qhT_h     (K=64 halves at partition offsets
#                                        0/64 -> PE tile-concurrent pairs)
#   ex = exp(logitsT/8); ex *= (1-mask)^T in place     (ACT then DVE)
#   av_h[slot, q] = vh^T @ ex           (row 0 = denominator, 64:128 = out)
#   attnN16[pair][128, q] = packed av pair * (1/denom broadcast across
#       partitions via SBUF->SBUF DMA of the reciprocal row)
#   partial[q, D] = attnN16^T @ wo_pair (K=128 pair-packed matmuls)
#
# Emission order: q proj (c-outer, all 8 PSUM banks), v proj, then 4
# attention blocks; k proj (qs-outer) is interleaved into block 0 so exp
# starts as early as possible.  Per steady block: WO of the previous block
# in slots kc3..6, AV pops 2/kc from kc8, mask prefetch at kc8/12,
# normalization at the block boundary.
import os
import numpy as np

B, S, DM, H, DEPTH = 2, 2048, 1024, 16, 64
NCORES = 8
GROUPS = 4            # head-groups per batch == cores per batch
HG = H // GROUPS      # heads per core
FS = HG * DEPTH       # features per core
QC = 512              # q-block (matmul free dim)
NQC = S // QC
NKC = S // 128        # k chunks
PAIRS = HG // 2
CCH = DM // 128       # contraction chunks for the projections

_CACHE = {}


def _build():
    import concourse.tile as tile
    from concourse import bacc, mybir

    dt = mybir.dt
    f32, f16 = dt.float32, dt.float16
    Act = mybir.ActivationFunctionType

    nc = bacc.Bacc("TRN2", target_bir_lowering=False, debug=False,
                   num_devices=NCORES)

    xq = nc.dram_tensor("xq", [DM, S], f16, kind="ExternalInput").ap()
    xk = nc.dram_tensor("xk", [DM, S], f16, kind="ExternalInput").ap()
    xv = nc.dram_tensor("xv", [DM, S], f16, kind="ExternalInput").ap()
    wqd = nc.dram_tensor("wq", [DM, FS], f16, kind="ExternalInput").ap()
    wkd = nc.dram_tensor("wk", [DM, FS], f16, kind="ExternalInput").ap()
    wvd = nc.dram_tensor("wv", [DM, FS], f16, kind="ExternalInput").ap()
    wod = nc.dram_tensor("wo", [PAIRS, 128, DM], f16, kind="ExternalInput").ap()
    m01 = nc.dram_tensor("m01", [S, S], f16, kind="ExternalInput").ap()
    bqd = nc.dram_tensor("bq", [128, 2], f32, kind="ExternalInput").ap()
    bkd = nc.dram_tensor("bk", [128, 2], f32, kind="ExternalInput").ap()
    out = nc.dram_tensor("part", [S, DM], f32, kind="ExternalOutput").ap()

    with tile.TileContext(nc) as tc:
        with (
            tc.tile_pool(name="xq", bufs=2) as xqp,       # q-proj x transient
            tc.tile_pool(name="xr", bufs=16) as xrp,      # xk + xv resident
            tc.tile_pool(name="wp", bufs=16) as wp,       # weight chunks
            tc.tile_pool(name="wop", bufs=1) as wop,
            tc.tile_pool(name="qk", bufs=4) as qkp,
            tc.tile_pool(name="vh", bufs=16) as vp,
            tc.tile_pool(name="mk", bufs=3) as mkp,       # mask half-tiles
            tc.tile_pool(name="ex", bufs=16) as exp_p,
            tc.tile_pool(name="an", bufs=4) as anp,
            tc.tile_pool(name="rr", bufs=2) as rrp,
            tc.tile_pool(name="rb", bufs=2) as rbp,
            tc.tile_pool(name="os", bufs=3) as osp,
            tc.tile_pool(name="cst", bufs=4) as cst,
            tc.tile_pool(name="ps2", bufs=2, space="PSUM") as ps2,   # 2-bank
            tc.tile_pool(name="ps1", bufs=4, space="PSUM") as ps1,   # 1-bank
        ):
            def big():
                return ps2.tile([128, 2, QC], f32, tag="big", name="big")

            def bank():
                return ps1.tile([128, QC], f32, tag="bank", name="bank")

            # ---- input DMAs ----
            # sync queue carries the q/k critical path (plus in-loop masks
            # and output stores); the scalar HWDGE queue carries the v/wo/
            # mask0 stream concurrently.
            def chunk_dma(pool, tag, dram, c, eng):
                t = pool.tile([128, S], f16, tag=tag, name=tag)
                eng.dma_start(t[:], dram[128 * c:128 * (c + 1), :])
                return t

            # mask half-tiles: [128, 8, QC] covering kc 8*h .. 8*h+7
            def mask_dma(qcb, h, eng):
                t = mkp.tile([128, NKC // 2, QC], f16, tag="mk", name="mk")
                eng.dma_start(
                    t[:],
                    m01[S // 2 * h:S // 2 * (h + 1), QC * qcb:QC * (qcb + 1)]
                    .rearrange("(kc p) q -> p kc q", p=128))
                return t

            wq_t, wk_t, xk_t, xv_t = [], [], [], []
            # sync stream: q path (weights first -- the first matmul needs
            # wq c0 + xq c0 only)
            w0 = wp.tile([128, FS], f16, tag="w", name="w")
            nc.sync.dma_start(w0[:], wqd[0:128, :])
            wq_t.append(w0)
            xq_pend = [chunk_dma(xqp, "xq", xq, c, nc.sync) for c in range(2)]
            for c in range(1, CCH):
                wt = wp.tile([128, FS], f16, tag="w", name="w")
                nc.sync.dma_start(wt[:], wqd[128 * c:128 * (c + 1), :])
                wq_t.append(wt)
            bq_t = cst.tile([128, 2], f32, tag="bias", name="bias")
            nc.sync.dma_start(bq_t[:], bqd[:])
            # gpsimd SWDGE stream: k path, concurrent with sync + scalar
            for ck in range(CCH):
                wt = wp.tile([128, FS], f16, tag="w", name="w")
                nc.gpsimd.dma_start(wt[:], wkd[128 * ck:128 * (ck + 1), :])
                wk_t.append(wt)
                xk_t.append(chunk_dma(xrp, "xr", xk, ck, nc.gpsimd))
            bk_t = cst.tile([128, 2], f32, tag="bias", name="bias")
            nc.gpsimd.dma_start(bk_t[:], bkd[:])
            # scalar-queue stream: first mask, then the v path + wo
            mk_cur = [mask_dma(0, 0, nc.scalar), mask_dma(0, 1, nc.scalar)]
            mk_next = [None, None]
            wv_t = wp.tile([128, CCH, FS], f16, tag="wv", name="wv", bufs=1)
            nc.scalar.dma_start(wv_t[:], wvd.rearrange("(c p) f -> p c f",
                                                       p=128))
            wo_t = wop.tile([128, PAIRS, DM], f16, tag="wo", name="wo")
            nc.scalar.dma_start(wo_t[:], wod.rearrange("h p d -> p h d"))
            for c in range(CCH):
                xv_t.append(chunk_dma(xrp, "xr", xv, c, nc.scalar))

            # ---- q projection (c-outer; transposed [FS, S] fp16 out) ----
            qhT = [qkp.tile([128, S], f16, tag="qk", name="qk")
                   for _ in range(2)]
            khT = [qkp.tile([128, S], f16, tag="qk", name="qk")
                   for _ in range(2)]

            # accumulators: 2 big (qs 0,1 x m) + 4 bank (qs 2,3 x m)
            pb = [big(), big()]
            ps = [bank() for _ in range(4)]

            def q_target(m, qs):
                if qs < 2:
                    return pb[qs][:, m, :]
                return ps[2 * (qs - 2) + m][:]

            for c in range(CCH):
                xt = xq_pend[c] if c < 2 else chunk_dma(xqp, "xq", xq, c,
                                                        nc.sync)
                for m in range(2):
                    for qs in range(NQC):
                        nc.tensor.matmul(
                            q_target(m, qs),
                            lhsT=wq_t[c][:, 128 * m:128 * (m + 1)],
                            rhs=xt[:, QC * qs:QC * (qs + 1)],
                            start=(c == 0), stop=(c == CCH - 1))
            for qs in range(NQC):
                for m in range(2):
                    nc.scalar.add(
                        qhT[m][:, QC * qs:QC * (qs + 1)],
                        q_target(m, qs), bq_t[:, m:m + 1])

            # ---- v projection, interleaved into block 0 (see below) ----
            vh = []

            def proj_v_kr(kr):
                pv = bank()
                for c in range(CCH):
                    nc.tensor.matmul(
                        pv[:, 0:FS],
                        lhsT=xv_t[c][:, 128 * kr:128 * (kr + 1)],
                        rhs=wv_t[:, c, :],
                        start=(c == 0), stop=(c == CCH - 1))
                t = vp.tile([128, HG, 128], f16, tag="vh", name="vh")
                nc.vector.memset(t[:, :, 0:1], 1.0)
                nc.vector.memset(t[:, :, 1:64], 0.0)
                nc.vector.tensor_copy(
                    t[:, :, 64:128],
                    pv[:, 0:FS].rearrange("p (h d) -> p h d", d=DEPTH))
                vh.append(t)

            # ---- attention ----
            def proj_k_qs(qs):
                p = big()
                for c in range(CCH):
                    for m in range(2):
                        nc.tensor.matmul(
                            p[:, m, :],
                            lhsT=wk_t[c][:, 128 * m:128 * (m + 1)],
                            rhs=xk_t[c][:, QC * qs:QC * (qs + 1)],
                            start=(c == 0), stop=(c == CCH - 1))
                for m in range(2):
                    nc.vector.tensor_scalar_add(
                        khT[m][:, QC * qs:QC * (qs + 1)],
                        p[:, m, :], bk_t[:, m:m + 1])

            def emit_logits(qcb, kc, pair):
                qsl = slice(QC * qcb, QC * (qcb + 1))
                ksl = slice(128 * kc, 128 * (kc + 1))
                lg2 = big()
                for half in range(2):
                    psl = slice(64 * half, 64 * (half + 1))
                    nc.tensor.matmul(
                        lg2[:, half, :],
                        lhsT=khT[pair][psl, ksl],
                        rhs=qhT[pair][psl, qsl],
                        start=True, stop=True)
                ex2 = exp_p.tile([128, 2, QC], f16, tag="ex", name="ex")
                nc.scalar.activation(ex2[:], lg2[:], Act.Exp, scale=0.125)
                nc.vector.tensor_mul(
                    ex2[:], ex2[:],
                    mk_cur[kc // 8][:, kc % 8:kc % 8 + 1, :]
                    .to_broadcast((128, 2, QC)))
                return ex2

            def emit_av(av2, pair, dk, ex2):
                # av2: [tile_half0, tile_half1]; vh head = 2*pair+half
                for half in range(2):
                    nc.tensor.matmul(
                        av2[half][:],
                        lhsT=vh[dk][:, 2 * pair + half, :],
                        rhs=ex2[:, half, :],
                        start=(dk == 0), stop=(dk == NKC - 1),
                        skip_group_check=True)

            def normalize_pair(av2, p, last=False):
                # drain a pair's av PSUM banks fast, then broadcast 1/denom
                # across partitions on the (otherwise idle) GpSimd.  On the
                # final block the copies go to ACT (its exp stream is done).
                cpy = nc.scalar.copy if last else nc.vector.tensor_copy
                at = anp.tile([128, QC], f16, tag="an", name="an")
                rt = rrp.tile([1, 2, QC], f32, tag="rr", name="rr")
                for half in range(2):
                    cpy(at[64 * half:64 * (half + 1), :],
                        av2[half][64:128, :])
                    nc.vector.reciprocal_approx_fast(
                        rt[0:1, half, :], av2[half][0:1, :])
                rba = rbp.tile([128, QC], f32, tag="rb", name="rb")
                rbb = rbp.tile([128, QC], f32, tag="rb", name="rb")
                nc.gpsimd.partition_broadcast(rba[:], rt[0:1, 0, :])
                nc.gpsimd.partition_broadcast(rbb[:], rt[0:1, 1, :])
                nc.vector.tensor_mul(at[0:64, :], at[0:64, :],
                                     rba[0:64, :])
                nc.vector.tensor_mul(at[64:128, :], at[64:128, :],
                                     rbb[64:128, :])
                return at

            def emit_wo(pq, attnN, qm):
                row = slice(128 * (4 * pq + qm), 128 * (4 * pq + qm + 1))
                po = [bank(), bank()]
                for dn in range(2):
                    for p in range(PAIRS):
                        nc.tensor.matmul(
                            po[dn][:],
                            lhsT=attnN[p][:, 128 * qm:128 * (qm + 1)],
                            rhs=wo_t[:, p, 512 * dn:512 * (dn + 1)],
                            start=(p == 0), stop=(p == PAIRS - 1))
                ot = osp.tile([128, 2, QC], f32, tag="os", name="os")
                nc.scalar.copy(ot[:, 0, :], po[0][:])
                nc.vector.tensor_copy(ot[:, 1, :], po[1][:])
                nc.scalar.dma_start(
                    out[row, :].rearrange("p (o q) -> p o q", o=2), ot[:])

            # v-projection schedule inside block 0: kr chunks per kc slot
            # (xv lands on the scalar queue while q/k stream on sync)
            vsched = {3: 3, 4: 3, 5: 3, 6: 3, 7: 2, 8: 2}

            # ---- block 0: kc-major (k/v projections share the slots) ----
            prev = None
            av_h = None
            pend = {p: [] for p in range(PAIRS)}
            for kc in range(NKC):
                if kc % 4 == 0:
                    proj_k_qs(kc // 4)
                if kc >= 9:                   # after pv has left PSUM
                    if av_h is None:
                        av_h = [bank() for _ in range(HG)]
                    for _ in range(2):
                        for pair in range(PAIRS):
                            if pend[pair]:
                                dk, dex = pend[pair].pop(0)
                                emit_av(av_h[2 * pair:2 * pair + 2],
                                        pair, dk, dex)
                for pair in range(PAIRS):
                    ex2 = emit_logits(0, kc, pair)
                    pend[pair].append((kc, ex2))
                if kc in vsched:
                    for _ in range(vsched[kc]):
                        proj_v_kr(len(vh))
                if kc == 8:
                    mk_next[0] = mask_dma(1, 0, nc.sync)
                elif kc == 12:
                    mk_next[1] = mask_dma(1, 1, nc.sync)
            for pair in range(PAIRS):
                for dk, dex in pend[pair]:
                    emit_av(av_h[2 * pair:2 * pair + 2], pair, dk, dex)
            prev = (0, [normalize_pair(av_h[2 * p:2 * p + 2], p)
                        for p in range(PAIRS)])
            mk_cur = mk_next
            mk_next = [None, None]

            # ---- blocks 1..3: kc-major with AV pops trailing by ~7 ----
            for qcb in range(1, NQC):
                last = qcb == NQC - 1
                av_h = None
                pend = {p: [] for p in range(PAIRS)}
                for kc in range(NKC):
                    if kc >= 7:
                        if av_h is None:
                            av_h = [bank() for _ in range(HG)]
                        for _ in range(2):
                            for pair in range(PAIRS):
                                if pend[pair]:
                                    dk, dex = pend[pair].pop(0)
                                    emit_av(av_h[2 * pair:2 * pair + 2],
                                            pair, dk, dex)
                    for pair in range(PAIRS):
                        ex2 = emit_logits(qcb, kc, pair)
                        pend[pair].append((kc, ex2))
                    if 3 <= kc <= 6 and prev is not None:
                        pq, pattn = prev
                        emit_wo(pq, pattn, kc - 3)
                        if kc == 6:
                            prev = None
                    if not last:
                        if kc == 8:
                            mk_next[0] = mask_dma(qcb + 1, 0, nc.sync)
                        elif kc == 12:
                            mk_next[1] = mask_dma(qcb + 1, 1, nc.sync)
                for pair in range(PAIRS):
                    for dk, dex in pend[pair]:
                        emit_av(av_h[2 * pair:2 * pair + 2], pair, dk, dex)
                prev = (qcb, [normalize_pair(av_h[2 * p:2 * p + 2], p,
                                             last=last)
                              for p in range(PAIRS)])
                mk_cur = mk_next
                mk_next = [None, None]

            pq, attnN = prev
            for qm in range(4):
                emit_wo(pq, attnN, qm)

    nc.compile()
    return nc


def _get_program():
    if "nc" not in _CACHE:
        _CACHE["nc"] = _build()
    return _CACHE["nc"]


def _in_maps(q, k, v, mask, wq, bq, wk, bk, wv, bv, wo, bo):
    q = np.asarray(q, np.float32)
    k = np.asarray(k, np.float32)
    v = np.asarray(v, np.float32)
    mask = np.asarray(mask, np.float32)
    wq = np.asarray(wq, np.float32)
    wk = np.asarray(wk, np.float32)
    wv = np.asarray(wv, np.float32)
    wo = np.asarray(wo, np.float32)
    bq = np.asarray(bq, np.float32)
    bk = np.asarray(bk, np.float32)
    bv = np.asarray(bv, np.float32)
    assert np.all(bv == 0.0), "nonzero bv not supported by this kernel"

    maps = []
    xqT = [np.ascontiguousarray(q[b].T).astype(np.float16) for b in range(B)]
    xkT = [np.ascontiguousarray(k[b].T).astype(np.float16) for b in range(B)]
    xvT = [np.ascontiguousarray(v[b].T).astype(np.float16) for b in range(B)]
    m01 = [np.ascontiguousarray((1.0 - mask[b, 0]).T).astype(np.float16)
           for b in range(B)]
    for c in range(NCORES):
        b, g = divmod(c, GROUPS)
        cols = slice(FS * g, FS * (g + 1))
        # wo rows for this group's heads, pair-packed: [pair, (half, depth), DM]
        wog = np.ascontiguousarray(
            wo[:, cols].T.reshape(PAIRS, 128, DM)).astype(np.float16)
        maps.append({
            "xq": xqT[b], "xk": xkT[b], "xv": xvT[b],
            "wq": np.ascontiguousarray(wq[cols].T).astype(np.float16),
            "wk": np.ascontiguousarray(wk[cols].T).astype(np.float16),
            "wv": np.ascontiguousarray(wv[cols].T).astype(np.float16),
            "wo": wog,
            "m01": m01[b],
            "bq": np.ascontiguousarray(bq[cols].reshape(2, 128).T),
            "bk": np.ascontiguousarray(bk[cols].reshape(2, 128).T),
        })
    return maps


def _run(maps, trace=False):
    from concourse.bass_utils import run_bass_kernel_spmd
    nc = _get_program()
    kwargs = {}
    if trace:
        kwargs = dict(trace=True, tmpdir=os.environ.get("KERNEL_TRACE_DIR"))
    return run_bass_kernel_spmd(nc, maps, list(range(NCORES)), **kwargs)


def kernel(q, k, v, mask, wq, bq, wk, bk, wv, bv, wo, bo):
    maps = _in_maps(q, k, v, mask, wq, bq, wk, bk, wv, bv, wo, bo)
    res = _run(maps)
    parts = [res.results[c]["part"] for c in range(NCORES)]
    bo = np.asarray(bo, np.float32)
    outb = [parts[GROUPS * b] + parts[GROUPS * b + 1]
            + parts[GROUPS * b + 2] + parts[GROUPS * b + 3] + bo
            for b in range(B)]
    return np.stack(outb, 0).astype(np.float32)
